# revision 45
# baseline (speedup 1.0000x reference)
"""Trainium2 Bass kernel for KernelDWConv2d.

out[b,o,h,w] = sum_{c,i,j} x[b,c,h+i,w+j] * kern[b,c,i,j] * weight[o,c,i,j] + bias[o]

Strategy (8 cores, data-parallel over batch, 4 samples/core):
  - Fold kern into weight on VectorE per (i,j,c_half) tap:
        wm[c,o] = weightT[c,(i,j),o] * kern[b,c,(i,j)]   (bf16 out)
  - Contract with TensorE bf16 matmuls (1 col/cycle, FWL weight loads):
        psum[o, hw] += wm[:,o].T @ x[c, h+i, w+j]
    The x windows are read straight out of SBUF with strided APs — no
    patch materialization, no column padding (N=325/300 exactly).
  - 98 K-tiles (49 taps x 2 c-halves) accumulate into 4 PSUM banks
    (2 o-halves x 2 row-splits of the 25x25 output).
  - Head/tail overlap: weight preload DMAs issue on the scalar hwdge
    queue so the sample-0 input DMAs on the sync queue aren't stuck
    behind them; the first-needed pieces (kern, x halves, weight tap 0)
    land first; dummy matmuls on a zero tile keep the PE busy during
    the initial DMA wait so the HAM clock gate is open (2.4 GHz) when
    the real matmuls start; sample 0 leads with ch0-only K-tiles (the
    smallest DMA dependency set); the drain splits across VectorE +
    ScalarE and both DMA queues.
"""

import sys

import numpy as np

if "/opt/trn_rl_repo" not in sys.path:
    sys.path.insert(0, "/opt/trn_rl_repo")

B, C, O, K, H, W = 32, 256, 256, 7, 31, 31
HO = WO = 25
NPIX = HO * WO  # 625
NCORES = 8
BPC = B // NCORES  # 4 samples per core
NTAP = K * K  # 49
# output row split: rows [0,13) -> N=325, rows [13,25) -> N=300 (<=512 fp32/bank)
ROW_SPLITS = [(0, 13), (13, 12)]
WCHUNK = 7  # taps per weight-DMA chunk
NWARM = 10  # dummy matmuls (N=325) to warm the PE clock during DMA wait

_STATE = {}


def _build_nc():
    if "nc" in _STATE:
        return _STATE["nc"]

    import concourse.bass as bass
    import concourse.bacc as bacc
    import concourse.mybir as mybir
    import concourse.tile as tile

    f32 = mybir.dt.float32
    bf16 = mybir.dt.bfloat16
    IDENT = mybir.ActivationFunctionType.Identity

    nc = bacc.Bacc("TRN2")

    xs_d = nc.dram_tensor("xs", [BPC, 128, 2 * H * W], bf16, kind="ExternalInput")
    kn_d = nc.dram_tensor("kern", [BPC, 128, 2, NTAP], f32, kind="ExternalInput")
    # group 0 (taps 0-6) separately, laid [128, 7*O] so it can be fetched in
    # two pieces (tap 0 alone = 64KB, the rest) to unblock the first matmul
    wT0_d = nc.dram_tensor("wT0", [2, 128, WCHUNK * O], bf16, kind="ExternalInput")
    wT_d = nc.dram_tensor("wT", [2, NTAP // WCHUNK - 1, 128, WCHUNK * O], bf16, kind="ExternalInput")
    bias_d = nc.dram_tensor("bias", [2, 128, 1], f32, kind="ExternalInput")
    out_d = nc.dram_tensor("out", [BPC, 2, 128, NPIX], bf16, kind="ExternalOutput")

    with tile.TileContext(nc) as tc:
        with (
            tc.tile_pool(name="wpool", bufs=1) as wpool,
            tc.tile_pool(name="xpool", bufs=2) as xpool,
            tc.tile_pool(name="wmpool", bufs=4) as wmpool,
            tc.tile_pool(name="opool", bufs=4) as opool,
            tc.tile_pool(name="pspool", bufs=2, space=bass.MemorySpace.PSUM) as pspool,
        ):
            # both channel-halves contiguous (single DMA descriptor per
            # partition); window slices read up to 6 elements past each
            # half (values never used), so only the tail needs real padding
            XSPAN = max(
                (i + r0) * W + j + nr * W
                for i in range(K)
                for j in range(K)
                for (r0, nr) in ROW_SPLITS
            )  # 967
            XTILE = H * W + XSPAN  # 1928

            # PE warm-up: matmuls on a zeroed tile, no DMA deps, so the PE
            # clock un-throttles while the first inputs stream in. The psum
            # tile matches the real ps00 tiles in tag+shape so pool bank
            # allocation stays consistent.
            warm_sb = wpool.tile([128, 325], bf16, tag="warm")
            nc.gpsimd.memset(warm_sb[:], 0)
            ps_warm = pspool.tile([128, 325], f32, tag="ps00", name="ps_warm")
            for _ in range(NWARM):
                nc.tensor.matmul(
                    ps_warm[:], warm_sb[:, 0:128], warm_sb[:], start=True, stop=True
                )

            def fetch_sample(b, first=False):
                xt = xpool.tile([128, XTILE], bf16, tag="x", name=f"x_{b}")
                kt_ = xpool.tile([128, 2 * NTAP], f32, tag="k", name=f"k_{b}")
                if first:
                    # kern is tiny and gates the first fold: land it first.
                    # x stays on the sync queue so the scalar queue carries
                    # only weights (the K-tile 1..13 dependency chain).
                    nc.sync.dma_start(out=kt_[:], in_=kn_d[b])
                    nc.sync.dma_start(out=xt[:, 0 : H * W], in_=xs_d[b][:, 0 : H * W])
                    nc.sync.dma_start(
                        out=xt[:, H * W : 2 * H * W], in_=xs_d[b][:, H * W :]
                    )
                else:
                    # one contiguous DMA for both channel-halves of x: one
                    # big descriptor per partition
                    nc.sync.dma_start(out=xt[:, 0 : 2 * H * W], in_=xs_d[b])
                    nc.sync.dma_start(out=kt_[:], in_=kn_d[b])
                x_t = [xt[:, 0:XSPAN], xt[:, H * W : H * W + XSPAN]]
                k_t = [kt_[:, 0:NTAP], kt_[:, NTAP : 2 * NTAP]]
                return x_t, k_t

            # Weight preload issues on the scalar hwdge queue: the first
            # taps are needed immediately and stream in small pieces, the
            # rest streams behind without blocking the per-sample input
            # DMAs on the sync queue.
            wt_t = {}

            def fetch_wt0(ch, pertap):
                t = wpool.tile(
                    [128, WCHUNK * O], bf16, tag=f"wT{ch}_0", name=f"wT{ch}_0"
                )
                if pertap:
                    # tap 0 alone (64KB, unblocks the first fold fast),
                    # then taps 1-6 as one well-packed transfer
                    nc.scalar.dma_start(out=t[:, 0:O], in_=wT0_d[ch][:, 0:O])
                    nc.scalar.dma_start(out=t[:, O:], in_=wT0_d[ch][:, O:])
                else:
                    nc.scalar.dma_start(out=t[:], in_=wT0_d[ch])
                wt_t[(ch, 0)] = t

            def fetch_wt(ch, g):
                t = wpool.tile(
                    [128, WCHUNK * O], bf16, tag=f"wT{ch}_{g}", name=f"wT{ch}_{g}"
                )
                nc.scalar.dma_start(out=t[:], in_=wT_d[ch, g - 1])
                wt_t[(ch, g)] = t

            fetch_wt0(0, pertap=True)
            sample0 = fetch_sample(0, first=True)
            fetch_wt0(1, pertap=True)
            for g in range(1, NTAP // WCHUNK):
                for ch in range(2):
                    fetch_wt(ch, g)
            bias_t = []
            for oh in range(2):
                t = wpool.tile([128, 1], f32, tag=f"bias{oh}")
                nc.scalar.dma_start(out=t[:], in_=bias_d[oh])
                bias_t.append(t)

            for b in range(BPC):
                x_t, k_t = sample0 if b == 0 else fetch_sample(b)

                ps = [
                    [
                        pspool.tile(
                            [128, nr * WO], f32, tag=f"ps{oh}{nh}", name=f"ps{oh}{nh}"
                        )
                        for nh, (r0, nr) in enumerate(ROW_SPLITS)
                    ]
                    for oh in range(2)
                ]

                # K-tile order: for sample 0, lead with the ch0 taps of the
                # first weight chunk (they only need k0+x0+wt00, the first
                # DMAs to land), then the ch1 taps, then interleave. Other
                # samples interleave throughout. Accumulation order is free.
                if b == 0:
                    ktiles = (
                        [(ij, 0) for ij in range(WCHUNK)]
                        + [(ij, 1) for ij in range(WCHUNK)]
                        + [(ij, ch) for ij in range(WCHUNK, NTAP) for ch in range(2)]
                    )
                else:
                    ktiles = [(ij, ch) for ij in range(NTAP) for ch in range(2)]

                n_k = 2 * NTAP  # 98
                for kt_idx, (ij, ch) in enumerate(ktiles):
                    i, j = divmod(ij, K)
                    wm = wmpool.tile([128, O], bf16, tag="wm")
                    nc.vector.tensor_scalar_mul(
                        wm[:],
                        wt_t[(ch, ij // WCHUNK)][
                            :, (ij % WCHUNK) * O : (ij % WCHUNK + 1) * O
                        ],
                        k_t[ch][:, ij : ij + 1],
                    )
                    for oh in range(2):
                        lhsT = wm[:, oh * 128 : (oh + 1) * 128]
                        for nh, (r0, nr) in enumerate(ROW_SPLITS):
                            off = (i + r0) * W + j
                            rhs = x_t[ch][:, off : off + nr * W].rearrange(
                                "p (r c) -> p r c", r=nr, c=W
                            )[:, :, 0:WO]
                            nc.tensor.matmul(
                                ps[oh][nh][:],
                                lhsT,
                                rhs,
                                start=(kt_idx == 0),
                                stop=(kt_idx == n_k - 1),
                            )

                for oh in range(2):
                    for nh, (r0, nr) in enumerate(ROW_SPLITS):
                        n = nr * WO
                        ot = opool.tile([128, n], bf16, tag=f"ot{nh}")
                        if oh == 1 and nh == 1:
                            # last bank drains on ScalarE in parallel with
                            # VectorE doing the other three
                            nc.scalar.activation(
                                ot[:], ps[oh][nh][:], IDENT, bias=bias_t[oh][:]
                            )
                        else:
                            nc.vector.tensor_scalar_add(
                                ot[:], ps[oh][nh][:], bias_t[oh][:]
                            )
                        # alternate issue queues so the tail's 4 issues
                        # don't serialize on one hwdge queue
                        dma_eng = nc.sync if nh == 0 else nc.scalar
                        dma_eng.dma_start(
                            out=out_d[b, oh, :, r0 * WO : r0 * WO + n], in_=ot[:]
                        )

    nc.finalize()
    _STATE["nc"] = nc
    return nc


def run(inputs, trace=False):
    import ml_dtypes
    from concourse.bass_utils import run_bass_kernel_spmd

    nc = _build_nc()
    bf16 = ml_dtypes.bfloat16

    x = np.asarray(inputs["x"], dtype=np.float32)
    kern = np.asarray(inputs["kernel"], dtype=np.float32)
    weight = np.asarray(inputs["weight"], dtype=np.float32)
    bias = np.asarray(inputs["bias"], dtype=np.float32)

    xs = (
        np.ascontiguousarray(
            x.reshape(NCORES, BPC, 2, 128, H * W).transpose(0, 1, 3, 2, 4)
        )
        .astype(bf16)
        .reshape(NCORES, BPC, 128, 2 * H * W)
    )
    ks = np.ascontiguousarray(
        kern.reshape(NCORES, BPC, 2, 128, NTAP).transpose(0, 1, 3, 2, 4)
    )  # [8, BPC, 128, 2, NTAP]
    wTg = np.ascontiguousarray(
        weight.transpose(1, 2, 3, 0).reshape(2, 128, NTAP // WCHUNK, WCHUNK * O).transpose(0, 2, 1, 3)
    ).astype(bf16)  # [2, 7, 128, 7*O]
    wT0 = np.ascontiguousarray(wTg[:, 0])  # [2, 128, 7*O]
    wT = np.ascontiguousarray(wTg[:, 1:])  # [2, 6, 128, 7*O]
    bs = np.ascontiguousarray(bias.reshape(2, 128, 1))

    in_maps = [
        {
            "xs": np.ascontiguousarray(xs[c]),
            "kern": np.ascontiguousarray(ks[c]),
            "wT0": wT0,
            "wT": wT,
            "bias": bs,
        }
        for c in range(NCORES)
    ]

    res = run_bass_kernel_spmd(nc, in_maps, list(range(NCORES)), trace=trace)
    out = np.stack(
        [np.asarray(res.results[c]["out"], dtype=np.float32) for c in range(NCORES)]
    )  # [8,4,2,128,625]
    out = out.reshape(B, O, HO, WO)
    return out, res


def kernel(**inputs):
    out, _ = run(inputs, trace=False)
    return out


# revision 46
# speedup vs baseline: 1.0087x; 1.0087x over previous
"""Trainium2 Bass kernel for KernelDWConv2d.

out[b,o,h,w] = sum_{c,i,j} x[b,c,h+i,w+j] * kern[b,c,i,j] * weight[o,c,i,j] + bias[o]

Strategy (8 cores, data-parallel over batch, 4 samples/core):
  - Fold kern into weight on VectorE per (i,j,c_half) tap:
        wm[c,o] = weightT[c,(i,j),o] * kern[b,c,(i,j)]   (bf16 out)
  - Contract with TensorE bf16 matmuls (1 col/cycle, FWL weight loads):
        psum[o, hw] += wm[:,o].T @ x[c, h+i, w+j]
    The x windows are read straight out of SBUF with strided APs — no
    patch materialization, no column padding (N=325/300 exactly).
  - 98 K-tiles (49 taps x 2 c-halves) accumulate into 4 PSUM banks
    (2 o-halves x 2 row-splits of the 25x25 output).
  - Head/tail overlap: weight preload DMAs issue on the scalar hwdge
    queue so the sample-0 input DMAs on the sync queue aren't stuck
    behind them; the first-needed pieces (kern, x halves, weight tap 0)
    land first; dummy matmuls on a zero tile keep the PE busy during
    the initial DMA wait so the HAM clock gate is open (2.4 GHz) when
    the real matmuls start; sample 0 leads with ch0-only K-tiles (the
    smallest DMA dependency set); the drain splits across VectorE +
    ScalarE and both DMA queues.
"""

import sys

import numpy as np

if "/opt/trn_rl_repo" not in sys.path:
    sys.path.insert(0, "/opt/trn_rl_repo")

B, C, O, K, H, W = 32, 256, 256, 7, 31, 31
HO = WO = 25
NPIX = HO * WO  # 625
NCORES = 8
BPC = B // NCORES  # 4 samples per core
NTAP = K * K  # 49
# output row split: rows [0,13) -> N=325, rows [13,25) -> N=300 (<=512 fp32/bank)
ROW_SPLITS = [(0, 13), (13, 12)]
WCHUNK = 7  # taps per weight-DMA chunk
NWARM = 14  # dummy matmuls (N=325) to warm the PE clock during DMA wait

_STATE = {}


def _build_nc():
    if "nc" in _STATE:
        return _STATE["nc"]

    import concourse.bass as bass
    import concourse.bacc as bacc
    import concourse.mybir as mybir
    import concourse.tile as tile

    f32 = mybir.dt.float32
    bf16 = mybir.dt.bfloat16
    IDENT = mybir.ActivationFunctionType.Identity

    nc = bacc.Bacc("TRN2")

    xs_d = nc.dram_tensor("xs", [BPC, 128, 2 * H * W], bf16, kind="ExternalInput")
    kn_d = nc.dram_tensor("kern", [BPC, 128, 2, NTAP], f32, kind="ExternalInput")
    # group 0 (taps 0-6) separately, laid [128, 7*O] so it can be fetched in
    # two pieces (tap 0 alone = 64KB, the rest) to unblock the first matmul
    wT0_d = nc.dram_tensor("wT0", [2, 128, WCHUNK * O], bf16, kind="ExternalInput")
    wT_d = nc.dram_tensor("wT", [2, NTAP // WCHUNK - 1, 128, WCHUNK * O], bf16, kind="ExternalInput")
    bias_d = nc.dram_tensor("bias", [2, 128, 1], f32, kind="ExternalInput")
    out_d = nc.dram_tensor("out", [BPC, 2, 128, NPIX], bf16, kind="ExternalOutput")

    with tile.TileContext(nc) as tc:
        with (
            tc.tile_pool(name="wpool", bufs=1) as wpool,
            tc.tile_pool(name="xpool", bufs=2) as xpool,
            tc.tile_pool(name="wmpool", bufs=4) as wmpool,
            tc.tile_pool(name="opool", bufs=4) as opool,
            tc.tile_pool(name="pspool", bufs=2, space=bass.MemorySpace.PSUM) as pspool,
        ):
            # both channel-halves contiguous (single DMA descriptor per
            # partition); window slices read up to 6 elements past each
            # half (values never used), so only the tail needs real padding
            XSPAN = max(
                (i + r0) * W + j + nr * W
                for i in range(K)
                for j in range(K)
                for (r0, nr) in ROW_SPLITS
            )  # 967
            XTILE = H * W + XSPAN  # 1928

            # PE warm-up: matmuls on a zeroed tile, no DMA deps, so the PE
            # clock un-throttles while the first inputs stream in. The psum
            # tile matches the real ps00 tiles in tag+shape so pool bank
            # allocation stays consistent.
            warm_sb = wpool.tile([128, 325], bf16, tag="warm")
            nc.gpsimd.memset(warm_sb[:], 0)
            ps_warm = pspool.tile([128, 325], f32, tag="ps00", name="ps_warm")
            for _ in range(NWARM):
                nc.tensor.matmul(
                    ps_warm[:], warm_sb[:, 0:128], warm_sb[:], start=True, stop=True
                )

            def fetch_sample(b, first=False):
                xt = xpool.tile([128, XTILE], bf16, tag="x", name=f"x_{b}")
                kt_ = xpool.tile([128, 2 * NTAP], f32, tag="k", name=f"k_{b}")
                if first:
                    # kern is tiny and gates the first fold: land it first.
                    # x stays on the sync queue so the scalar queue carries
                    # only weights (the K-tile 1..13 dependency chain).
                    nc.sync.dma_start(out=kt_[:], in_=kn_d[b])
                    nc.sync.dma_start(out=xt[:, 0 : H * W], in_=xs_d[b][:, 0 : H * W])
                    nc.sync.dma_start(
                        out=xt[:, H * W : 2 * H * W], in_=xs_d[b][:, H * W :]
                    )
                else:
                    # one contiguous DMA for both channel-halves of x: one
                    # big descriptor per partition
                    nc.sync.dma_start(out=xt[:, 0 : 2 * H * W], in_=xs_d[b])
                    nc.sync.dma_start(out=kt_[:], in_=kn_d[b])
                x_t = [xt[:, 0:XSPAN], xt[:, H * W : H * W + XSPAN]]
                k_t = [kt_[:, 0:NTAP], kt_[:, NTAP : 2 * NTAP]]
                return x_t, k_t

            # Weight preload issues on the scalar hwdge queue: the first
            # taps are needed immediately and stream in small pieces, the
            # rest streams behind without blocking the per-sample input
            # DMAs on the sync queue.
            wt_t = {}

            def fetch_wt0(ch, pertap):
                t = wpool.tile(
                    [128, WCHUNK * O], bf16, tag=f"wT{ch}_0", name=f"wT{ch}_0"
                )
                if pertap:
                    # tap 0 alone (64KB, unblocks the first fold fast),
                    # then taps 1-6 as one well-packed transfer
                    nc.scalar.dma_start(out=t[:, 0:O], in_=wT0_d[ch][:, 0:O])
                    nc.scalar.dma_start(out=t[:, O:], in_=wT0_d[ch][:, O:])
                else:
                    nc.scalar.dma_start(out=t[:], in_=wT0_d[ch])
                wt_t[(ch, 0)] = t

            def fetch_wt(ch, g):
                t = wpool.tile(
                    [128, WCHUNK * O], bf16, tag=f"wT{ch}_{g}", name=f"wT{ch}_{g}"
                )
                nc.scalar.dma_start(out=t[:], in_=wT_d[ch, g - 1])
                wt_t[(ch, g)] = t

            fetch_wt0(0, pertap=True)
            sample0 = fetch_sample(0, first=True)
            fetch_wt0(1, pertap=True)
            for g in range(1, NTAP // WCHUNK):
                for ch in range(2):
                    fetch_wt(ch, g)
            bias_t = []
            for oh in range(2):
                t = wpool.tile([128, 1], f32, tag=f"bias{oh}")
                nc.scalar.dma_start(out=t[:], in_=bias_d[oh])
                bias_t.append(t)

            for b in range(BPC):
                x_t, k_t = sample0 if b == 0 else fetch_sample(b)

                ps = [
                    [
                        pspool.tile(
                            [128, nr * WO], f32, tag=f"ps{oh}{nh}", name=f"ps{oh}{nh}"
                        )
                        for nh, (r0, nr) in enumerate(ROW_SPLITS)
                    ]
                    for oh in range(2)
                ]

                # K-tile order: for sample 0, lead with the ch0 taps of the
                # first weight chunk (they only need k0+x0+wt00, the first
                # DMAs to land), then the ch1 taps, then interleave. Other
                # samples interleave throughout. Accumulation order is free.
                if b == 0:
                    ktiles = (
                        [(ij, 0) for ij in range(WCHUNK)]
                        + [(ij, 1) for ij in range(WCHUNK)]
                        + [(ij, ch) for ij in range(WCHUNK, NTAP) for ch in range(2)]
                    )
                else:
                    ktiles = [(ij, ch) for ij in range(NTAP) for ch in range(2)]

                n_k = 2 * NTAP  # 98
                for kt_idx, (ij, ch) in enumerate(ktiles):
                    i, j = divmod(ij, K)
                    wm = wmpool.tile([128, O], bf16, tag="wm")
                    nc.vector.tensor_scalar_mul(
                        wm[:],
                        wt_t[(ch, ij // WCHUNK)][
                            :, (ij % WCHUNK) * O : (ij % WCHUNK + 1) * O
                        ],
                        k_t[ch][:, ij : ij + 1],
                    )
                    for oh in range(2):
                        lhsT = wm[:, oh * 128 : (oh + 1) * 128]
                        for nh, (r0, nr) in enumerate(ROW_SPLITS):
                            off = (i + r0) * W + j
                            rhs = x_t[ch][:, off : off + nr * W].rearrange(
                                "p (r c) -> p r c", r=nr, c=W
                            )[:, :, 0:WO]
                            nc.tensor.matmul(
                                ps[oh][nh][:],
                                lhsT,
                                rhs,
                                start=(kt_idx == 0),
                                stop=(kt_idx == n_k - 1),
                            )

                for oh in range(2):
                    for nh, (r0, nr) in enumerate(ROW_SPLITS):
                        n = nr * WO
                        ot = opool.tile([128, n], bf16, tag=f"ot{nh}")
                        if oh == 1 and nh == 1:
                            # last bank drains on ScalarE in parallel with
                            # VectorE doing the other three
                            nc.scalar.activation(
                                ot[:], ps[oh][nh][:], IDENT, bias=bias_t[oh][:]
                            )
                        else:
                            nc.vector.tensor_scalar_add(
                                ot[:], ps[oh][nh][:], bias_t[oh][:]
                            )
                        # alternate issue queues so the tail's 4 issues
                        # don't serialize on one hwdge queue
                        dma_eng = nc.sync if nh == 0 else nc.scalar
                        dma_eng.dma_start(
                            out=out_d[b, oh, :, r0 * WO : r0 * WO + n], in_=ot[:]
                        )

    nc.finalize()
    _STATE["nc"] = nc
    return nc


def run(inputs, trace=False):
    import ml_dtypes
    from concourse.bass_utils import run_bass_kernel_spmd

    nc = _build_nc()
    bf16 = ml_dtypes.bfloat16

    x = np.asarray(inputs["x"], dtype=np.float32)
    kern = np.asarray(inputs["kernel"], dtype=np.float32)
    weight = np.asarray(inputs["weight"], dtype=np.float32)
    bias = np.asarray(inputs["bias"], dtype=np.float32)

    xs = (
        np.ascontiguousarray(
            x.reshape(NCORES, BPC, 2, 128, H * W).transpose(0, 1, 3, 2, 4)
        )
        .astype(bf16)
        .reshape(NCORES, BPC, 128, 2 * H * W)
    )
    ks = np.ascontiguousarray(
        kern.reshape(NCORES, BPC, 2, 128, NTAP).transpose(0, 1, 3, 2, 4)
    )  # [8, BPC, 128, 2, NTAP]
    wTg = np.ascontiguousarray(
        weight.transpose(1, 2, 3, 0).reshape(2, 128, NTAP // WCHUNK, WCHUNK * O).transpose(0, 2, 1, 3)
    ).astype(bf16)  # [2, 7, 128, 7*O]
    wT0 = np.ascontiguousarray(wTg[:, 0])  # [2, 128, 7*O]
    wT = np.ascontiguousarray(wTg[:, 1:])  # [2, 6, 128, 7*O]
    bs = np.ascontiguousarray(bias.reshape(2, 128, 1))

    in_maps = [
        {
            "xs": np.ascontiguousarray(xs[c]),
            "kern": np.ascontiguousarray(ks[c]),
            "wT0": wT0,
            "wT": wT,
            "bias": bs,
        }
        for c in range(NCORES)
    ]

    res = run_bass_kernel_spmd(nc, in_maps, list(range(NCORES)), trace=trace)
    out = np.stack(
        [np.asarray(res.results[c]["out"], dtype=np.float32) for c in range(NCORES)]
    )  # [8,4,2,128,625]
    out = out.reshape(B, O, HO, WO)
    return out, res


def kernel(**inputs):
    out, _ = run(inputs, trace=False)
    return out
